# revision 10
# baseline (speedup 1.0000x reference)
"""Bass/Trainium2 kernel for attention-energy softmax:
  proj = enc @ W.T + b        [S,B,D]
  energies[b,s] = hidden[b] . proj[s,b]
  out = softmax(energies, axis=s)[:, None, :]

Algebraic fusion: energies[b,s] = (hidden[b] @ W) . enc[s,b] + hidden[b].b
The bias term is constant per b and cancels in softmax, so it is dropped.

PE-centric, fp16-streaming design (quantization rel-l2 ~2.4e-3 vs the
2e-2 gate; fp16 products are exact in the PE's f32 accumulation):

  1. Host casts enc/W/hidden to fp16 and pre-transposes enc so the
     contraction dim d lies on SBUF partitions. Each (sblk, half) chunk
     is one 4MB DMA (32KB contiguous per partition) on the sync HWDGE
     ring, issued from t=0; W/hidden ride the scalar ring concurrently.
  2. v = hidden @ W on the PE (fp16), then PE-transposed into
     vT[128d, g, 8b] fp16 stationaries.
  3. Energies: per d-chunk g, 8 matmuls with stationary vT[:, g, :]
     ([128, 8]) and moving enc[:, gg, b, :] ([128, 512]); PSUM bank b
     accumulates over the 8 g's. Row b of bank b is e[b, sblk block];
     other rows are cross-batch garbage that costs nothing (PE time is
     moving-column driven). Banks drain [0:8] to SBUF (engines cannot
     address off-quadrant partition bases) and a tiny SBUF-to-SBUF DMA
     on the scalar ring pulls the diagonal row into e_sb[b].
  4. Streaming block softmax: per seq block, exp(e - blockmax) + block
     sums run on ACT as soon as the block's rows land; the tail only
     combines (exp of blockmax deltas, one reciprocal, 4 scaled
     multiplies) and DMAs the output from its natural [BL, S] layout.
"""

import numpy as np

import concourse.bass as bass
import concourse.mybir as mybir
from concourse import bacc
from concourse.masks import make_identity
from concourse.bass_utils import run_bass_kernel_spmd
from concourse.tile import TileContext

S, B, D = 2048, 64, 1024
NCORES = 8
BL = B // NCORES  # 8 local batches per core
P = 128
NSB = 4       # seq blocks of 512
NCHUNK = NSB * 2  # one chunk = (sblk, half): 4 d-chunks x 8 b x 512 s = 4MB
F32 = mybir.dt.float32
F16 = mybir.dt.float16

TRACE = False  # test.py sets True to profile

_CACHE = {}


def build_kernel() -> bass.Bass:
    nc = bacc.Bacc(None, target_bir_lowering=False)
    enc_d = nc.dram_tensor("enc", [NCHUNK, P, 4, BL, 512], F16, kind="ExternalInput")
    # W[e, d] pre-permuted to [ee, ech, d] so e-chunk ech lives on partitions
    w_d = nc.dram_tensor("w", [P, 8, D], F16, kind="ExternalInput")
    # hidden.T pre-permuted to [ee, ech, b]
    h_d = nc.dram_tensor("h", [P, 8, BL], F16, kind="ExternalInput")
    out_d = nc.dram_tensor("out", [BL, S], F16, kind="ExternalOutput")

    with TileContext(nc) as tc:
        with (
            tc.tile_pool(name="consts", bufs=1) as consts,
            tc.tile_pool(name="encp", bufs=3) as encp,
            tc.tile_pool(name="scrp", bufs=2) as scrp,
            tc.tile_pool(name="ps", bufs=1, space="PSUM") as psp,
        ):
            # enc stream starts immediately on the sync ring
            ets = []
            for c in range(3):
                et = encp.tile([P, 4, BL, 512], F16, tag="e", name=f"et{c}")
                nc.sync.dma_start(out=et, in_=enc_d[c])
                ets.append(et)

            # prologue weights ride the scalar HWDGE ring concurrently
            hT_sb = consts.tile([P, 8, BL], F16)
            nc.scalar.dma_start(out=hT_sb, in_=h_d[:, :, :])
            W_sb = consts.tile([P, 8, D], F16)
            nc.scalar.dma_start(out=W_sb[:, 0:4, :], in_=w_d[:, 0:4, :])
            nc.scalar.dma_start(out=W_sb[:, 4:8, :], in_=w_d[:, 4:8, :])

            ident = consts.tile([P, P], F32)
            make_identity(nc, ident)
            # Warm the PE p-state while the weight DMAs are in flight.
            warm = psp.tile([P, 512], F32, tag="b7")
            for _ in range(8):
                nc.tensor.matmul(
                    warm[:, 0:P], ident, ident, start=True, stop=True
                )

            # ---- v = hidden @ W -> [BL, D] f32 in PSUM ----
            v_ps0 = psp.tile([P, 512], F32, tag="b0")
            v_ps1 = psp.tile([P, 512], F32, tag="b1")
            for ech in range(8):
                for half, v_ps in ((0, v_ps0), (1, v_ps1)):
                    nc.tensor.matmul(
                        v_ps[0:BL, :],
                        hT_sb[:, ech, :],
                        W_sb[:, ech, half * 512 : (half + 1) * 512],
                        start=(ech == 0),
                        stop=(ech == 7),
                    )
            v_sb = consts.tile([BL, D], F32)
            nc.scalar.copy(out=v_sb[:, 0:512], in_=v_ps0[0:BL, :])
            nc.scalar.copy(out=v_sb[:, 512:1024], in_=v_ps1[0:BL, :])

            # Preload the Exp table on ACT so block exps don't pay for it.
            dm1 = consts.tile([BL, 1], F32)
            nc.scalar.activation(
                out=dm1, in_=v_sb[:, 0:1], func=mybir.ActivationFunctionType.Exp
            )

            # ---- vT[dd, g, b] fp16 stationaries via PE transpose ----
            vT = consts.tile([P, 8, BL], F16)
            for g in range(8):
                tr = psp.tile([P, 512], F32, tag=f"b{2 + (g % 4)}")
                nc.tensor.transpose(
                    tr[:, 0:BL], v_sb[:, g * P : (g + 1) * P], ident[0:BL, 0:BL]
                )
                nc.scalar.copy(out=vT[:, g, :], in_=tr[:, 0:BL])

            # ---- main loop: stream enc, accumulate energies in PSUM ----
            e_sb = consts.tile([BL, S], F32)
            p_sb = consts.tile([BL, S], F32)
            pm = consts.tile([BL, NSB], F32)
            npm = consts.tile([BL, NSB], F32)
            s4 = consts.tile([BL, NSB], F32)
            for sblk in range(NSB):
                ps = [
                    psp.tile([P, 512], F32, tag=f"b{b}", name=f"ps{b}")
                    for b in range(BL)
                ]
                # engines cannot address single partitions off quadrant
                # bases, so drain each bank [0:8] into scr and pull the
                # one valid row (the diagonal) with a tiny SBUF-to-SBUF
                # DMA on the scalar HWDGE ring.
                scr = scrp.tile([BL, BL, 512], F32, tag="scr")
                for half in range(2):
                    c = sblk * 2 + half
                    if c < 3:
                        et = ets[c]
                    else:
                        et = encp.tile([P, 4, BL, 512], F16, tag="e", name=f"et{c}")
                        if c < NCHUNK - 1:
                            nc.sync.dma_start(out=et, in_=enc_d[c])
                        else:
                            # split the last chunk so the PE drains a short tail
                            for gg in range(4):
                                nc.sync.dma_start(
                                    out=et[:, gg], in_=enc_d[c, :, gg]
                                )
                    for gg in range(4):
                        g = 4 * half + gg
                        for b in range(BL):
                            nc.tensor.matmul(
                                ps[b][0:BL, :],
                                vT[:, g, :],
                                et[:, gg, b, :],
                                start=(g == 0),
                                stop=(g == 7),
                            )
                            if g == 7:
                                # drain bank b as soon as its accumulation ends
                                eng = (
                                    nc.scalar.copy
                                    if b % 2 == 0
                                    else nc.vector.tensor_copy
                                )
                                eng(out=scr[:, b, :], in_=ps[b][0:BL, :])
                                # diagonal row extraction on otherwise-idle
                                # DMA rings (gpsimd SWDGE mid-stream; the
                                # sync ring joins for the final block once
                                # the enc stream is done)
                                dma = (
                                    nc.sync.dma_start
                                    if (sblk == NSB - 1 and b % 2 == 1)
                                    else nc.gpsimd.dma_start
                                )
                                dma(
                                    out=e_sb[
                                        b : b + 1, sblk * 512 : (sblk + 1) * 512
                                    ],
                                    in_=scr[b : b + 1, b, :],
                                )
                # streaming block softmax: blockmax, then exp+sum on ACT
                blk = slice(sblk * 512, (sblk + 1) * 512)
                nc.vector.tensor_reduce(
                    out=pm[:, sblk : sblk + 1],
                    in_=e_sb[:, blk],
                    axis=mybir.AxisListType.X,
                    op=mybir.AluOpType.max,
                )
                nc.vector.tensor_scalar_mul(
                    npm[:, sblk : sblk + 1], pm[:, sblk : sblk + 1], -1.0
                )
                nc.scalar.activation(
                    out=p_sb[:, blk],
                    in_=e_sb[:, blk],
                    func=mybir.ActivationFunctionType.Exp,
                    bias=npm[:, sblk : sblk + 1],
                    accum_out=s4[:, sblk : sblk + 1],
                )

            # ---- combine: out_k = p_k * exp(m_k - M) / sum_j s_j e^{m_j-M}
            m1 = consts.tile([BL, 1], F32)
            nc.vector.tensor_reduce(
                out=m1, in_=pm, axis=mybir.AxisListType.X, op=mybir.AluOpType.max
            )
            nm1 = consts.tile([BL, 1], F32)
            nc.vector.tensor_scalar_mul(nm1, m1, -1.0)
            w4 = consts.tile([BL, NSB], F32)
            nc.scalar.activation(
                out=w4,
                in_=pm,
                func=mybir.ActivationFunctionType.Exp,
                bias=nm1,
            )
            sw = consts.tile([BL, 1], F32)
            dms = consts.tile([BL, NSB], F32)
            nc.vector.scalar_tensor_tensor(
                out=dms,
                in0=s4,
                scalar=1.0,
                in1=w4,
                op0=mybir.AluOpType.mult,
                op1=mybir.AluOpType.mult,
                accum_out=sw,
            )
            r1 = consts.tile([BL, 1], F32)
            nc.vector.reciprocal(r1, sw)
            coef = consts.tile([BL, NSB], F32)
            nc.vector.tensor_scalar_mul(coef, w4, r1[:, 0:1])
            o_sb = consts.tile([BL, S], F16)
            for sblk in range(NSB):
                blk = slice(sblk * 512, (sblk + 1) * 512)
                nc.vector.tensor_scalar_mul(
                    o_sb[:, blk], p_sb[:, blk], coef[:, sblk : sblk + 1]
                )
            nc.sync.dma_start(out=out_d[:, :], in_=o_sb)

    nc.compile()
    return nc


def _stage_inputs(hidden, encoder_outputs, W_attn):
    h16 = np.asarray(hidden, dtype=np.float32)[0].astype(np.float16)  # [B, D]
    W16 = np.asarray(W_attn, dtype=np.float32).astype(np.float16)
    # W[e, d] -> [ee, ech, d]
    w_stage = np.ascontiguousarray(W16.reshape(8, P, D).transpose(1, 0, 2))
    enc16 = np.asarray(encoder_outputs, dtype=np.float32).astype(np.float16)
    # One big transpose to [d, b, s]; per-core reorders then hit contiguous
    # runs of 512 in the fast axis.
    encT = np.ascontiguousarray(enc16.transpose(2, 1, 0))  # [D, B, S]

    in_maps = []
    for c in range(NCORES):
        bs = slice(c * BL, (c + 1) * BL)
        # hT[ee, ech, b] = h[b, ech*128+ee]
        hT = np.ascontiguousarray(h16[bs].T.reshape(8, P, BL).transpose(1, 0, 2))
        # enc_t[sblk, half, dd, gg, b, ss] = encT[(4half+gg)*128+dd, b, sblk*512+ss]
        ec = encT[:, bs, :].reshape(2, 4, P, BL, NSB, 512)
        ect = np.ascontiguousarray(ec.transpose(4, 0, 2, 1, 3, 5)).reshape(
            NCHUNK, P, 4, BL, 512
        )
        in_maps.append({"enc": ect, "w": w_stage, "h": hT})
    return in_maps


def kernel(hidden, encoder_outputs, W_attn, b_attn):
    in_maps = _stage_inputs(hidden, encoder_outputs, W_attn)

    if "nc" not in _CACHE:
        _CACHE["nc"] = build_kernel()
    nc = _CACHE["nc"]

    res = run_bass_kernel_spmd(nc, in_maps, core_ids=list(range(NCORES)), trace=TRACE)
    if TRACE:
        _CACHE["last_result"] = res
    out = np.concatenate([r["out"] for r in res.results], axis=0)  # [B, S] fp16
    return out[:, None, :].astype(np.float32)
